# revision 17
# baseline (speedup 1.0000x reference)
"""Trainium2 Bass kernel: 15x15 valid cross-correlation on a 4096x4096 fp32 image.

Strategy
--------
- Host shards the image rows across 8 NeuronCores (halo exchange done by
  overlapping host slices; band matrices replicated).  Each core computes
  456 output rows (4 row-blocks of 114) over all 4082 columns, plus a
  512-wide column stripe of the remaining 434 output rows (rows
  3648..4081), so the remainder work is spread evenly across cores.
- On each core the conv runs on TensorE as band-Toeplitz matmuls: for an
  output row-block of M=114 rows (input rows c0..c0+127) and an output
  column tile [j0, j0+N):
      psum[:, 0:N] += B_q.T @ X[c0:c0+128, j0+q : j0+q+N]   for q = 0..14
  where B_q[c, i] = w[c - i, q] is a 128x128 band matrix prepacked on the
  host.  The 15 matmuls accumulate in one PSUM bank (float32r: fp32 data
  at bf16 stream rate); VectorE evacuates PSUM; DMA writes the output.
- A short burst of dummy matmuls at kernel start warms the PE HAM clock
  gate while the first DMAs are still streaming.
"""

import numpy as np

H = 4096
W = 4096
KH = 15
KW = 15
OH = H - KH + 1  # 4082
OW = W - KW + 1  # 4082
NCORES = 8
BLOCK = 114  # output rows per row-block (128 - 15 + 1)
NTILE = 512  # output cols per matmul (one PSUM bank of fp32)

RPC = 4 * BLOCK  # 456 main output rows per core
STRIP_H = RPC + KH - 1  # 470 input rows per main strip
REM0 = NCORES * RPC  # 3648: first remainder output row
REM_ROWS = OH - REM0  # 434 remainder output rows
REM_IN_ROWS = H - REM0  # 448 input rows covering the remainder
XR_W = 528  # remainder stripe input width (512 + 14, padded to 528)

# Matmul compute dtype: "float32" (exact, 4 cyc/row), "float32r" (1 cyc/row),
# "bfloat16" or "float16" (1 cyc/row, lower precision).
COMPUTE_DT = "float32r"
WARMUP_MM = 12  # dummy matmuls to warm the PE clock gate

_BUILD_CACHE = {}


def _np_dtype(dt_name):
    if dt_name in ("float32", "float32r"):
        return np.float32
    if dt_name == "float16":
        return np.float16
    if dt_name == "bfloat16":
        import ml_dtypes

        return np.dtype(ml_dtypes.bfloat16)
    raise ValueError(dt_name)


def _build(dt_name):
    """Build the single-core Bass program (same program runs SPMD on 8 cores)."""
    import concourse.mybir as mybir
    from concourse import bacc
    from concourse.tile import TileContext

    store_dt = {
        "float32": mybir.dt.float32,
        "float32r": mybir.dt.float32r,
        "float16": mybir.dt.float16,
        "bfloat16": mybir.dt.bfloat16,
    }[dt_name]

    # Bacc (not plain Bass): its finalize() runs move_matmul_waits_to_ldweights
    # + generate_event_semaphores, without which walrus rejects matmuls that
    # carry more than one semaphore wait ("Too many sync wait commands").
    nc = bacc.Bacc(None, target_bir_lowering=False)
    x_d = nc.dram_tensor("x", [STRIP_H, W], store_dt, kind="ExternalInput")
    xr_d = nc.dram_tensor("xr", [REM_IN_ROWS, XR_W], store_dt, kind="ExternalInput")
    b_d = nc.dram_tensor("bands", [128, KW * 128], store_dt, kind="ExternalInput")
    o_d = nc.dram_tensor("out", [RPC, OW], mybir.dt.float32, kind="ExternalOutput")
    or_d = nc.dram_tensor(
        "outr", [REM_ROWS, NTILE], mybir.dt.float32, kind="ExternalOutput"
    )

    with TileContext(nc) as tc:
        with (
            tc.tile_pool(name="bands", bufs=1) as bpool,
            tc.tile_pool(name="x", bufs=2) as xpool,
            tc.tile_pool(name="xr", bufs=2) as xrpool,
            tc.tile_pool(name="pswarm", bufs=1, space="PSUM") as pswpool,
            tc.tile_pool(name="ps", bufs=6, space="PSUM") as pspool,
            tc.tile_pool(name="o", bufs=4) as opool,
        ):
            # PE warmup: a small fast DMA feeds dummy matmuls so the HAM
            # clock gate reaches 8/8 while the big input DMAs stream.
            wsb = bpool.tile([128, NTILE], store_dt, tag="warm")
            nc.sync.dma_start(wsb[:], b_d[:, :NTILE])
            # single accumulation group: no inter-group PSUM hazards to stall on
            psw = pswpool.tile([128, NTILE], mybir.dt.float32)
            for i in range(WARMUP_MM):
                nc.tensor.matmul(
                    psw[:],
                    wsb[:, :128],
                    wsb[:],
                    start=(i == 0),
                    stop=(i == WARMUP_MM - 1),
                )

            bsb = bpool.tile([128, KW * 128], store_dt)
            nc.sync.dma_start(bsb[:], b_d[:])

            def conv_block(src_tile, kb, mv, j0, nt, dst, dr0):
                """One accumulation group: 15 matmuls + evacuate + store."""
                ps = pspool.tile([128, NTILE], mybir.dt.float32)
                for q in range(KW):
                    nc.tensor.matmul(
                        ps[:, :nt],
                        bsb[:kb, q * 128 : (q + 1) * 128],
                        src_tile[:kb, j0 + q : j0 + q + nt],
                        start=(q == 0),
                        stop=(q == KW - 1),
                    )
                ot = opool.tile([128, NTILE], mybir.dt.float32)
                nc.vector.tensor_copy(ot[:mv, :nt], ps[:mv, :nt])
                nc.sync.dma_start(dst[dr0 : dr0 + mv, j0 : j0 + nt], ot[:mv, :nt])

            # 4 rounds; each round = one main row-block (8 column tiles)
            # followed by one remainder-stripe row-block (1 tile), so the PE
            # never sees a long idle phase.  Stripe DMAs are prefetched one
            # round ahead so the stripe block never waits on DMA.
            def load_stripe(b):
                r0 = b * BLOCK
                kbr = min(128, REM_IN_ROWS - r0)
                xrt = xrpool.tile([128, XR_W], store_dt, tag=f"xr{b % 2}")
                nc.sync.dma_start(xrt[:kbr, :], xr_d[r0 : r0 + kbr, :])
                return xrt, kbr

            for b in range(4):
                r0 = b * BLOCK
                kb = min(128, STRIP_H - r0)
                xt = xpool.tile([128, W], store_dt)
                # column-halved DMA so early tiles start sooner
                half = W // 2 + KW - 1
                nc.sync.dma_start(xt[:kb, :half], x_d[r0 : r0 + kb, :half])
                nc.sync.dma_start(xt[:kb, half:], x_d[r0 : r0 + kb, half:])
                for j0 in range(0, OW, NTILE):
                    nt = min(NTILE, OW - j0)
                    conv_block(xt, kb, min(BLOCK, RPC - r0), j0, nt, o_d, r0)
                    if j0 == 0:
                        stripe = load_stripe(b)

                # remainder stripe block (per-core column stripe data)
                xrt, kbr = stripe
                mvr = min(BLOCK, REM_ROWS - r0)
                conv_block(xrt, kbr, mvr, 0, NTILE, or_d, r0)
    nc.finalize()
    return nc


def _make_bands(weight, np_dt):
    """bands[c, q*128 + i] = w[c - i, q] for 0 <= c - i < KH."""
    bands = np.zeros((128, KW * 128), dtype=np.float32)
    for q in range(KW):
        for d in range(KH):
            i = np.arange(0, 128 - d)
            bands[i + d, q * 128 + i] = weight[d, q]
    return bands.astype(np_dt)


def kernel(X, weight, bias):
    import os

    from concourse.bass_utils import run_bass_kernel_spmd

    X = np.asarray(X, dtype=np.float32)
    weight = np.asarray(weight, dtype=np.float32)
    bias = np.asarray(bias, dtype=np.float32)

    np_dt = _np_dtype(COMPUTE_DT)

    if COMPUTE_DT not in _BUILD_CACHE:
        _BUILD_CACHE[COMPUTE_DT] = _build(COMPUTE_DT)
    nc = _BUILD_CACHE[COMPUTE_DT]

    xc = X.astype(np_dt)
    bands = _make_bands(weight, np_dt)

    # Remainder stripe inputs: rows [3648, 4096), 528-wide column windows.
    xr_full = np.zeros((REM_IN_ROWS, NCORES * NTILE + KW - 1 + 2), dtype=np_dt)
    xr_full[:, :W] = xc[REM0:]

    in_maps = []
    for c in range(NCORES):
        in_maps.append(
            {
                "x": np.ascontiguousarray(xc[RPC * c : RPC * c + STRIP_H]),
                "xr": np.ascontiguousarray(
                    xr_full[:, NTILE * c : NTILE * c + XR_W]
                ),
                "bands": bands,
            }
        )

    trace = bool(int(os.environ.get("CONV_KERNEL_TRACE", "0")))
    tracedir = os.environ.get("CONV_KERNEL_TRACE_DIR") or None
    res = run_bass_kernel_spmd(
        nc, in_maps, core_ids=list(range(NCORES)), trace=trace, tmpdir=tracedir
    )

    out = np.empty((OH, OW), dtype=np.float32)
    for c in range(NCORES):
        out[RPC * c : RPC * (c + 1)] = res.results[c]["out"]
    for c in range(NCORES):
        w_c = min(NTILE, OW - NTILE * c)
        out[REM0:, NTILE * c : NTILE * c + w_c] = res.results[c]["outr"][:, :w_c]
    if res.exec_time_ns is not None:
        print(f"HW exec time: {res.exec_time_ns} ns")
    return (out + bias[0]).astype(np.float32)


# revision 19
# speedup vs baseline: 1.0294x; 1.0294x over previous
"""Trainium2 Bass kernel: 15x15 valid cross-correlation on a 4096x4096 fp32 image.

Strategy
--------
- Host shards the image rows across 8 NeuronCores (halo exchange done by
  overlapping host slices; band matrices replicated).  Each core computes
  456 output rows (4 row-blocks of 114) over all 4082 columns, plus a
  512-wide column stripe of the remaining 434 output rows (rows
  3648..4081), so the remainder work is spread evenly across cores.
- On each core the conv runs on TensorE as band-Toeplitz matmuls: for an
  output row-block of M=114 rows (input rows c0..c0+127) and an output
  column tile [j0, j0+N):
      psum[:, 0:N] += B_q.T @ X[c0:c0+128, j0+q : j0+q+N]   for q = 0..14
  where B_q[c, i] = w[c - i, q] is a 128x128 band matrix prepacked on the
  host.  The 15 matmuls accumulate in one PSUM bank (float32r: fp32 data
  at bf16 stream rate); VectorE evacuates PSUM; DMA writes the output.
- A short burst of dummy matmuls at kernel start warms the PE HAM clock
  gate while the first DMAs are still streaming.
"""

import numpy as np

H = 4096
W = 4096
KH = 15
KW = 15
OH = H - KH + 1  # 4082
OW = W - KW + 1  # 4082
NCORES = 8
BLOCK = 114  # output rows per row-block (128 - 15 + 1)
NTILE = 512  # output cols per matmul (one PSUM bank of fp32)

RPC = 4 * BLOCK  # 456 main output rows per core
STRIP_H = RPC + KH - 1  # 470 input rows per main strip
REM0 = NCORES * RPC  # 3648: first remainder output row
REM_ROWS = OH - REM0  # 434 remainder output rows
REM_IN_ROWS = H - REM0  # 448 input rows covering the remainder
XR_W = 528  # remainder stripe input width (512 + 14, padded to 528)

# Matmul compute dtype: "float32" (exact, 4 cyc/row), "float32r" (1 cyc/row),
# "bfloat16" or "float16" (1 cyc/row, lower precision).
COMPUTE_DT = "float32r"
WARMUP_MM = 12  # dummy matmuls to warm the PE clock gate

_BUILD_CACHE = {}


def _np_dtype(dt_name):
    if dt_name in ("float32", "float32r"):
        return np.float32
    if dt_name == "float16":
        return np.float16
    if dt_name == "bfloat16":
        import ml_dtypes

        return np.dtype(ml_dtypes.bfloat16)
    raise ValueError(dt_name)


def _build(dt_name):
    """Build the single-core Bass program (same program runs SPMD on 8 cores)."""
    import concourse.mybir as mybir
    from concourse import bacc
    from concourse.tile import TileContext

    store_dt = {
        "float32": mybir.dt.float32,
        "float32r": mybir.dt.float32r,
        "float16": mybir.dt.float16,
        "bfloat16": mybir.dt.bfloat16,
    }[dt_name]

    # Bacc (not plain Bass): its finalize() runs move_matmul_waits_to_ldweights
    # + generate_event_semaphores, without which walrus rejects matmuls that
    # carry more than one semaphore wait ("Too many sync wait commands").
    nc = bacc.Bacc(None, target_bir_lowering=False)
    x_d = nc.dram_tensor("x", [STRIP_H, W], store_dt, kind="ExternalInput")
    xr_d = nc.dram_tensor("xr", [REM_IN_ROWS, XR_W], store_dt, kind="ExternalInput")
    b_d = nc.dram_tensor("bands", [128, KW * 128], store_dt, kind="ExternalInput")
    o_d = nc.dram_tensor("out", [RPC, OW], mybir.dt.float32, kind="ExternalOutput")
    or_d = nc.dram_tensor(
        "outr", [REM_ROWS, NTILE], mybir.dt.float32, kind="ExternalOutput"
    )

    with TileContext(nc) as tc:
        with (
            tc.tile_pool(name="bands", bufs=1) as bpool,
            tc.tile_pool(name="x", bufs=2) as xpool,
            tc.tile_pool(name="xr", bufs=2) as xrpool,
            tc.tile_pool(name="pswarm", bufs=1, space="PSUM") as pswpool,
            tc.tile_pool(name="ps", bufs=6, space="PSUM") as pspool,
            tc.tile_pool(name="o", bufs=4) as opool,
        ):
            # PE warmup: a small fast DMA feeds dummy matmuls so the HAM
            # clock gate reaches 8/8 while the big input DMAs stream.
            wsb = bpool.tile([128, NTILE], store_dt, tag="warm")
            nc.sync.dma_start(wsb[:], b_d[:, :NTILE])
            # single accumulation group: no inter-group PSUM hazards to stall on
            psw = pswpool.tile([128, NTILE], mybir.dt.float32)
            for i in range(WARMUP_MM):
                nc.tensor.matmul(
                    psw[:],
                    wsb[:, :128],
                    wsb[:],
                    start=(i == 0),
                    stop=(i == WARMUP_MM - 1),
                )

            bsb = bpool.tile([128, KW * 128], store_dt)
            nc.sync.dma_start(bsb[:], b_d[:])

            def conv_block(src_tile, kb, mv, j0, nt, dst, dr0):
                """One accumulation group: 15 matmuls + evacuate + store."""
                ps = pspool.tile([128, NTILE], mybir.dt.float32)
                for q in range(KW):
                    nc.tensor.matmul(
                        ps[:, :nt],
                        bsb[:kb, q * 128 : (q + 1) * 128],
                        src_tile[:kb, j0 + q : j0 + q + nt],
                        start=(q == 0),
                        stop=(q == KW - 1),
                    )
                ot = opool.tile([128, NTILE], mybir.dt.float32)
                nc.vector.tensor_copy(ot[:mv, :nt], ps[:mv, :nt])
                # output stores go via the scalar-engine HWDGE queue so they
                # never gate the input-load queue (nc.sync)
                nc.scalar.dma_start(dst[dr0 : dr0 + mv, j0 : j0 + nt], ot[:mv, :nt])

            # 4 rounds; each round = one main row-block (8 column tiles)
            # followed by one remainder-stripe row-block (1 tile), so the PE
            # never sees a long idle phase.  Stripe DMAs are prefetched one
            # round ahead so the stripe block never waits on DMA.
            def load_stripe(b):
                r0 = b * BLOCK
                kbr = min(128, REM_IN_ROWS - r0)
                xrt = xrpool.tile([128, XR_W], store_dt, tag=f"xr{b % 2}")
                nc.sync.dma_start(xrt[:kbr, :], xr_d[r0 : r0 + kbr, :])
                return xrt, kbr

            half = W // 2 + KW - 1

            def load_main(b):
                r0 = b * BLOCK
                kb = min(128, STRIP_H - r0)
                xt = xpool.tile([128, W], store_dt)
                # column-halved DMA so early tiles start sooner
                nc.sync.dma_start(xt[:kb, :half], x_d[r0 : r0 + kb, :half])
                nc.sync.dma_start(xt[:kb, half:], x_d[r0 : r0 + kb, half:])
                return xt, kb

            nxt = load_main(0)
            for b in range(4):
                r0 = b * BLOCK
                xt, kb = nxt
                for j0 in range(0, OW, NTILE):
                    nt = min(NTILE, OW - j0)
                    conv_block(xt, kb, min(BLOCK, RPC - r0), j0, nt, o_d, r0)
                    if j0 == 0:
                        stripe = load_stripe(b)
                    if j0 == NTILE and b < 3:
                        nxt = load_main(b + 1)

                # remainder stripe block (per-core column stripe data)
                xrt, kbr = stripe
                mvr = min(BLOCK, REM_ROWS - r0)
                conv_block(xrt, kbr, mvr, 0, NTILE, or_d, r0)
    nc.finalize()
    return nc


def _make_bands(weight, np_dt):
    """bands[c, q*128 + i] = w[c - i, q] for 0 <= c - i < KH."""
    bands = np.zeros((128, KW * 128), dtype=np.float32)
    for q in range(KW):
        for d in range(KH):
            i = np.arange(0, 128 - d)
            bands[i + d, q * 128 + i] = weight[d, q]
    return bands.astype(np_dt)


def kernel(X, weight, bias):
    import os

    from concourse.bass_utils import run_bass_kernel_spmd

    X = np.asarray(X, dtype=np.float32)
    weight = np.asarray(weight, dtype=np.float32)
    bias = np.asarray(bias, dtype=np.float32)

    np_dt = _np_dtype(COMPUTE_DT)

    if COMPUTE_DT not in _BUILD_CACHE:
        _BUILD_CACHE[COMPUTE_DT] = _build(COMPUTE_DT)
    nc = _BUILD_CACHE[COMPUTE_DT]

    xc = X.astype(np_dt)
    bands = _make_bands(weight, np_dt)

    # Remainder stripe inputs: rows [3648, 4096), 528-wide column windows.
    xr_full = np.zeros((REM_IN_ROWS, NCORES * NTILE + KW - 1 + 2), dtype=np_dt)
    xr_full[:, :W] = xc[REM0:]

    in_maps = []
    for c in range(NCORES):
        in_maps.append(
            {
                "x": np.ascontiguousarray(xc[RPC * c : RPC * c + STRIP_H]),
                "xr": np.ascontiguousarray(
                    xr_full[:, NTILE * c : NTILE * c + XR_W]
                ),
                "bands": bands,
            }
        )

    trace = bool(int(os.environ.get("CONV_KERNEL_TRACE", "0")))
    tracedir = os.environ.get("CONV_KERNEL_TRACE_DIR") or None
    res = run_bass_kernel_spmd(
        nc, in_maps, core_ids=list(range(NCORES)), trace=trace, tmpdir=tracedir
    )

    out = np.empty((OH, OW), dtype=np.float32)
    for c in range(NCORES):
        out[RPC * c : RPC * (c + 1)] = res.results[c]["out"]
    for c in range(NCORES):
        w_c = min(NTILE, OW - NTILE * c)
        out[REM0:, NTILE * c : NTILE * c + w_c] = res.results[c]["outr"][:, :w_c]
    if res.exec_time_ns is not None:
        print(f"HW exec time: {res.exec_time_ns} ns")
    return (out + bias[0]).astype(np.float32)
